# revision 17
# baseline (speedup 1.0000x reference)
"""Trainium2 Bass kernel: per-point 3x3 Gaussian covariance from quaternion + log_scale.

cov = R diag(exp(log_scale)) R^T with R built from the normalized quaternion.

Strategy (v5, planar fp16 + merged multi-AP instructions):
  * Host reshapes inputs to struct-of-arrays fp16 planes per core:
    q [128, 4, R], ls [128, 3, R]; device writes the 6 unique entries of the
    symmetric cov as fp16 planes [128, 6, R] (diag first); host mirrors/casts
    to [N,3,3] f32.
  * Math: with half-square sums x0=(a^2+b^2-c^2-d^2)/2 etc. and unnormalized
    rotation half-columns x=(x0, bc+ad, bd-ac), y=(bc-ad, y1, cd+ab):
        cov = s2*I + alpha * x x^T + beta * y y^T
    where alpha=(s0-s2)*4/n^4, beta=(s1-s2)*4/n^4 via inv4=exp(-2*ln(n^2/2)).
    Only TWO outer products thanks to sum_j r_j r_j^T = I.
  * Logical ops are packed into few wide DVE instructions using multi-dim
    access patterns (outer dims with arbitrary/zero/negative strides, unit
    inner stride keeps the 2x fp16 perf mode). 22 DVE + 4 ACT + 3 DMA
    instructions per tile.
"""

import os
import numpy as np

import concourse.bass as bass
import concourse.bacc as bacc
import concourse.mybir as mybir
from concourse.tile import TileContext
from concourse.bass_utils import run_bass_kernel_spmd

AF = mybir.ActivationFunctionType
OP = mybir.AluOpType
FP16 = mybir.dt.float16
FP32 = mybir.dt.float32

N_CORES = 8
N_FULL = 4_000_000
P = 128
R = 3912                      # rows per partition per core; 128*3912*8 >= 4M
NPC = P * R                   # points per core (padded)
F = int(os.environ.get("KERNEL_F", "1184"))   # main tile size
F0 = int(os.environ.get("KERNEL_F0", "360"))  # fill tile size
F1 = int(os.environ.get("KERNEL_F1", "400"))  # drain tile size
RECIP_MODE = os.environ.get("KERNEL_RECIP", "act")

SQRT_HALF = 0.7071067811865476

_built = {}


def _tile_schedule():
    """Small first/last tiles to shrink pipeline fill/drain."""
    head = [F0] if 0 < F0 < R else []
    tail = [F1] if 0 < F1 < R - (F0 if head else 0) else []
    sizes, rem = head, R - sum(head) - sum(tail)
    while rem > 0:
        fcur = min(F, rem)
        if fcur % 2:
            fcur += 1 if rem > fcur else -1
        sizes.append(min(fcur, rem))
        rem -= sizes[-1]
    return sizes + tail


def _build():
    key = (F, F0, F1, RECIP_MODE)
    if key in _built:
        return _built[key]

    nc = bacc.Bacc("TRN2", target_bir_lowering=False, debug=False, num_devices=N_CORES)
    q = nc.dram_tensor("q", [P, 4, R], FP16, kind="ExternalInput")
    ls = nc.dram_tensor("ls", [P, 3, R], FP16, kind="ExternalInput")
    cov = nc.dram_tensor("cov", [P, 6, R], FP16, kind="ExternalOutput")

    qv, lsv, ov = q.ap(), ls.ap(), cov.ap()

    with TileContext(nc) as tc:
        with (
            tc.tile_pool(name="io", bufs=2) as io,
            tc.tile_pool(name="otp", bufs=2) as otp,
            tc.tile_pool(name="wk2", bufs=2) as wk2,
            tc.tile_pool(name="wk1", bufs=1) as wk1,
        ):
            t0 = 0
            for f in _tile_schedule():
                _tile_body(nc, io, otp, wk2, wk1, qv, lsv, ov, t0, f)
                t0 += f

    nc.compile()
    _built[key] = nc
    return nc


def _tile_body(nc, io, otp, wk2, wk1, qv, lsv, ov, t0, f):
    V = nc.vector

    def rows(ap, c):
        return ap.rearrange("p (c f) -> p c f", c=c)

    def bcast(ap_f, n):
        # [P, f] -> [P, n, f] with zero stride on the middle dim
        return ap_f.unsqueeze(1).broadcast_to([P, n, f])

    qt = io.tile([P, 4 * f], FP16, tag="qt", name=f"qt{t0}")
    lst = io.tile([P, 3 * f], FP16, tag="lst", name=f"lst{t0}")
    nc.sync.dma_start(out=rows(qt, 4), in_=qv[:, :, t0:t0 + f])
    nc.sync.dma_start(out=rows(lst, 3), in_=lsv[:, :, t0:t0 + f])
    qr = rows(qt, 4)  # (a, b, c, d)

    # ---- ScalarE: squares and scale exps (one instruction each) ----------
    sq4 = wk2.tile([P, 4 * f], FP16, tag="sq4", name=f"sq4_{t0}")   # sa sb sc sd
    sexp = wk2.tile([P, 3 * f], FP16, tag="sexp", name=f"sexp{t0}")  # s0 s1 s2
    nc.scalar.activation(sq4, qt, AF.Square, scale=SQRT_HALF)
    nc.scalar.activation(sexp, lst, AF.Exp)
    sr = rows(sq4, 4)

    # ---- cross products: prod6 = (ab, ac, ad, bc, bd, cd) ----------------
    prod6 = wk1.tile([P, 6 * f], FP16, tag="prod6", name=f"prod6_{t0}")
    pr = rows(prod6, 6)
    V.tensor_mul(pr[:, 0:3, :], bcast(qt[:, 0:f], 3), qr[:, 1:4, :])   # ab ac ad
    V.tensor_mul(pr[:, 3:5, :], bcast(qt[:, f:2 * f], 2), qr[:, 2:4, :])  # bc bd
    V.tensor_mul(pr[:, 5:6, :], qr[:, 2:3, :], qr[:, 3:4, :])          # cd

    # ---- half-square combos: uvz = (u, v, u2', v2) -----------------------
    uvz = wk1.tile([P, 6 * f], FP16, tag="uvz", name=f"uvz{t0}")  # sized 6f for m1 reuse
    ur = rows(uvz, 6)
    V.tensor_add(ur[:, 0:2, :], sr[:, 0:3:2, :], sr[:, 1:4:2, :])  # u=sa+sb, v=sc+sd
    V.tensor_sub(ur[:, 2:4, :], sr[:, 1:3, :], sr[:, 0:4:3, :])    # u2'=sb-sa, v2=sc-sd
    n2h = wk2.tile([P, f], FP16, tag="n2h", name=f"n2h{t0}")
    V.tensor_add(n2h, uvz[:, 0:f], uvz[:, f:2 * f])

    # ---- rotation half-columns: xy6 = (x0, x1, x2, y0, y1, y2) -----------
    xy6 = wk1.tile([P, 6 * f], FP16, tag="xy6", name=f"xy6_{t0}")
    xr = rows(xy6, 6)
    V.tensor_sub(xr[:, 0:5:4, :], ur[:, 0:4:3, :], ur[:, 1:3, :])  # x0=u-v, y1=v2-u2'
    V.tensor_sub(xr[:, 2:4, :], pr[:, 4:2:-1, :], pr[:, 1:3, :])   # x2=bd-ac, y0=bc-ad
    V.tensor_add(xr[:, 1:6:4, :], pr[:, 3:6:2, :], pr[:, 2::-2, :])  # x1=bc+ad, y2=cd+ab

    # ---- inv4 = 4/n^4 = 1/(n2h^2), avoiding the Ln table-set swap --------
    inv4 = wk2.tile([P, f], FP16, tag="inv4", name=f"inv4_{t0}")
    if RECIP_MODE == "act":
        # all on ScalarE: 1/n2h^2 = Square(AbsRsqrt(Square(n2h)))
        n4h = wk2.tile([P, f], FP16, tag="n4h", name=f"n4h{t0}")
        rsq = wk2.tile([P, f], FP16, tag="rsq", name=f"rsq{t0}")
        nc.scalar.activation(n4h, n2h, AF.Square)
        nc.scalar.activation(rsq, n4h, AF.Abs_reciprocal_sqrt)
        nc.scalar.activation(inv4, rsq, AF.Square)
    else:
        # ACT Square (same table set as Exp), then the fast-reciprocal custom
        # DVE op directly in fp16 (the BITWISE_NOT seed acts on the pipe's
        # internal fp32 conversion, so 16-bit I/O keeps downstream ops at 2x).
        from concourse.dve_ops import (
            RECIP_APPROX_FAST_CONSTS,
            RECIPROCAL_APPROX_FAST,
        )
        n4h = wk2.tile([P, f], FP16, tag="n4h", name=f"n4h{t0}")
        nc.scalar.activation(n4h, n2h, AF.Square)
        rc = RECIP_APPROX_FAST_CONSTS
        V._custom_dve(RECIPROCAL_APPROX_FAST, out=inv4, in0=n4h,
                      s0=rc["s0"], s1=rc["s1"], imm2=rc["imm2"])

    # ---- alpha/beta ------------------------------------------------------
    dd = wk1.tile([P, 3 * f], FP16, tag="dd", name=f"dd{t0}")  # sized 3f for dtmp reuse
    V.tensor_sub(rows(dd[:, 0:2 * f], 2), rows(sexp[:, 0:2 * f], 2),
                 bcast(sexp[:, 2 * f:3 * f], 2))               # d0=s0-s2, d1=s1-s2
    ab2 = wk1.tile([P, 2 * f], FP16, tag="ab2", name=f"ab2_{t0}")
    V.tensor_mul(rows(ab2, 2), rows(dd[:, 0:2 * f], 2), bcast(inv4, 2))  # al, be

    # ---- weighted columns ------------------------------------------------
    w03 = wk1.tile([P, 3 * f], FP16, tag="w03", name=f"w03_{t0}")
    w13 = wk1.tile([P, 3 * f], FP16, tag="w13", name=f"w13_{t0}")
    V.tensor_mul(rows(w03, 3), bcast(ab2[:, 0:f], 3), xr[:, 0:3, :])
    V.tensor_mul(rows(w13, 3), bcast(ab2[:, f:2 * f], 3), xr[:, 3:6, :])

    # ---- gram entries: m = (m_00, m_11, m_22, m_01, m_02, m_12) ----------
    m0 = wk1.tile([P, 6 * f], FP16, tag="prod6", name=f"m0_{t0}")
    m1 = wk1.tile([P, 6 * f], FP16, tag="uvz", name=f"m1_{t0}")
    m0r, m1r = rows(m0, 6), rows(m1, 6)
    V.tensor_mul(m0r[:, 0:3, :], rows(w03, 3), xr[:, 0:3, :])          # diag
    V.tensor_mul(m0r[:, 3:5, :], bcast(w03[:, 0:f], 2), xr[:, 1:3, :])  # m01 m02
    V.tensor_mul(m0r[:, 5:6, :], rows(w03, 3)[:, 1:2, :], xr[:, 2:3, :])  # m12
    V.tensor_mul(m1r[:, 0:3, :], rows(w13, 3), xr[:, 3:6, :])
    V.tensor_mul(m1r[:, 3:5, :], bcast(w13[:, 0:f], 2), xr[:, 4:6, :])
    V.tensor_mul(m1r[:, 5:6, :], rows(w13, 3)[:, 1:2, :], xr[:, 5:6, :])

    # ---- cov planes (diag first) -----------------------------------------
    ot = otp.tile([P, 6 * f], FP16, tag="ot", name=f"ot{t0}")
    dtmp = wk1.tile([P, 3 * f], FP16, tag="dd", name=f"dtmp{t0}")
    V.tensor_add(rows(dtmp, 3), m0r[:, 0:3, :], m1r[:, 0:3, :])
    V.tensor_add(rows(ot[:, 0:3 * f], 3), rows(dtmp, 3),
                 bcast(sexp[:, 2 * f:3 * f], 3))                        # diag + s2
    V.tensor_add(rows(ot[:, 3 * f:6 * f], 3), m0r[:, 3:6, :], m1r[:, 3:6, :])

    nc.sync.dma_start(out=ov[:, :, t0:t0 + f], in_=rows(ot, 6))


def _pack_inputs(quaternion, log_scale):
    n = quaternion.shape[0]
    total = N_CORES * NPC
    qp = np.empty((total, 4), np.float16)
    lp = np.empty((total, 3), np.float16)
    qp[:n] = quaternion[:n]
    lp[:n] = log_scale[:n]
    if total > n:
        qp[n:] = np.array([1, 0, 0, 0], np.float16)
        lp[n:] = 0
    in_maps = []
    for i in range(N_CORES):
        sl = slice(i * NPC, (i + 1) * NPC)
        qc = np.ascontiguousarray(qp[sl].reshape(P, R, 4).transpose(0, 2, 1))
        lc = np.ascontiguousarray(lp[sl].reshape(P, R, 3).transpose(0, 2, 1))
        in_maps.append({"q": qc, "ls": lc})
    return in_maps


def _unpack_output(results, n):
    # device planes: [P, 6, R] fp16, order (c00, c11, c22, c01, c02, c12)
    planes = np.concatenate(
        [r["cov"].transpose(0, 2, 1).reshape(NPC, 6) for r in results], axis=0
    )[:n].astype(np.float32)
    out = np.empty((n, 3, 3), np.float32)
    out[:, 0, 0] = planes[:, 0]
    out[:, 1, 1] = planes[:, 1]
    out[:, 2, 2] = planes[:, 2]
    out[:, 0, 1] = planes[:, 3]; out[:, 1, 0] = planes[:, 3]
    out[:, 0, 2] = planes[:, 4]; out[:, 2, 0] = planes[:, 4]
    out[:, 1, 2] = planes[:, 5]; out[:, 2, 1] = planes[:, 5]
    return out


def kernel_with_stats(quaternion, log_scale, trace=False):
    quaternion = np.asarray(quaternion, dtype=np.float32)
    log_scale = np.asarray(log_scale, dtype=np.float32)
    n = quaternion.shape[0]
    nc = _build()
    in_maps = _pack_inputs(quaternion, log_scale)
    res = run_bass_kernel_spmd(nc, in_maps, core_ids=list(range(N_CORES)), trace=trace)
    out = _unpack_output(res.results, n)
    return out, res


def kernel(quaternion, log_scale):
    out, _ = kernel_with_stats(quaternion, log_scale, trace=False)
    return out
